# revision 5
# baseline (speedup 1.0000x reference)
"""Trainium2 Bass kernel for tanh-attention (nn_Attention_50362786513376).

reference:
  q = (x @ Wq.T) * dk^-0.5 ; k = x @ Wk.T ; v = x        (heads = 8, dk = 64)
  out = tanh(q k^T) v   per (batch, head),  merged back to [b, n, dim]

Sharding: 8 cores = 4 batches x 2 head-halves (4 heads per core).
Host pre-work (free, exact): transpose x[b] -> xT, slice v channels, slice +
scale + transpose weights. Device per core:
  Q^T = WqT.T @ xT, K^T = WkT.T @ xT          (fp32r matmuls, contraction c=512)
  per head pair p, i-half, j-tile:  S^T[j,i] = K^T.T Q^T  (row-packed pairs)
  tanh on ScalarE PSUM->SBUF (the throughput bottleneck: n^2*h*b/8 elements)
  out^T[d,i] += v[j,:].T @ tanh(S^T)          (accumulated in PSUM over j)
Host post-work: out[b,:,half] = outT.T
"""
import numpy as np

HEADS = 8
DK = 64
B = 4
N = 2048
DIM = 512
SCALE = DK ** (-0.5)
NCORES = 8
HALF = DIM // 2  # 256 channels per core (4 heads)

_built = None
TRACE = False
TRACE_KW = {}


def _build():
    from contextlib import ExitStack

    import concourse.tile as tile
    from concourse import bacc, mybir

    F32 = mybir.dt.float32
    F32R = mybir.dt.float32r
    Tanh = mybir.ActivationFunctionType.Tanh

    nc = bacc.Bacc("TRN2", target_bir_lowering=False, debug=False,
                   num_devices=NCORES)
    xT_ap = nc.dram_tensor("xT", [DIM, N], F32R, kind="ExternalInput").ap()
    xv_ap = nc.dram_tensor("xv", [N, HALF], F32R, kind="ExternalInput").ap()
    wqT_ap = nc.dram_tensor("wqT", [DIM, HALF], F32R, kind="ExternalInput").ap()
    wkT_ap = nc.dram_tensor("wkT", [DIM, HALF], F32R, kind="ExternalInput").ap()
    outT_ap = nc.dram_tensor("outT", [HALF, N], F32, kind="ExternalOutput").ap()

    NT = N // 512          # 4 t-chunks of 512
    NJ = N // 128          # 16 j-tiles

    with tile.TileContext(nc) as tc:
        with ExitStack() as ctx:
            const = ctx.enter_context(tc.tile_pool(name="const", bufs=1))
            qk_pool = ctx.enter_context(tc.tile_pool(name="qk", bufs=1))
            tanh_pool = ctx.enter_context(tc.tile_pool(name="tanh", bufs=4))
            stg_pool = ctx.enter_context(tc.tile_pool(name="stg", bufs=4))
            ps_S = ctx.enter_context(
                tc.tile_pool(name="ps_S", bufs=2, space="PSUM"))
            ps_acc = ctx.enter_context(
                tc.tile_pool(name="ps_acc", bufs=4, space="PSUM"))

            # ---- load inputs ----
            # xT [512, 2048] -> [128, 4*2048] (c-tile ct at cols ct*2048)
            xT_sb = const.tile([128, 4 * N], F32R)
            nc.sync.dma_start(xT_sb[:], xT_ap.rearrange("(a p) t -> p a t", p=128))
            # xv [2048, 256] -> 4 tiles [128, 4*256] (j-tile j: g=j//4, a=j%4)
            xv_sb = []
            for g in range(4):
                t = const.tile([128, 4 * HALF], F32R, tag=f"xv{g}", name=f"xv{g}")
                nc.sync.dma_start(
                    t[:], xv_ap[g * 512:(g + 1) * 512, :]
                    .rearrange("(a p) c -> p a c", p=128))
                xv_sb.append(t)
            # wqT/wkT [512, 256] -> [128, 4*256] (c-tile ct at cols ct*256)
            wq_sb = const.tile([128, 4 * HALF], F32R)
            nc.sync.dma_start(wq_sb[:], wqT_ap.rearrange("(a p) m -> p a m", p=128))
            wk_sb = const.tile([128, 4 * HALF], F32R)
            nc.sync.dma_start(wk_sb[:], wkT_ap.rearrange("(a p) m -> p a m", p=128))

            # ---- projections: QT/KT pair tiles [128, 2048] ----
            QT = [qk_pool.tile([128, N], F32R, tag=f"qt{p}", name=f"qt{p}")
                  for p in range(2)]
            KT = [qk_pool.tile([128, N], F32R, tag=f"kt{p}", name=f"kt{p}")
                  for p in range(2)]
            for dst, w_sb in ((QT, wq_sb), (KT, wk_sb)):
                for p in range(2):
                    for t4 in range(NT):
                        ps = ps_S.tile([128, 512], F32, tag="S", name="proj_ps")
                        for ct in range(4):
                            lhsT = w_sb[:, ct * HALF + p * 128:
                                        ct * HALF + (p + 1) * 128]
                            rhs = xT_sb[:, ct * N + t4 * 512:
                                        ct * N + t4 * 512 + 512]
                            nc.tensor.matmul(ps[:], lhsT, rhs,
                                             start=(ct == 0), stop=(ct == 3))
                        nc.vector.tensor_copy(dst[p][:, t4 * 512:(t4 + 1) * 512],
                                              ps[:])

            # ---- attention ----
            for p in range(2):
                stg = [stg_pool.tile([64, N], F32, tag="stg", name="stg")
                       for _ in range(2)]
                for ih in range(2):          # i-half: i cols ih*1024..+1024
                    acc = [[ps_acc.tile([64, 512], F32, tag="acc", name="acc")
                            for ic in range(2)] for par in range(2)]
                    for j in range(NJ):
                        for ic in range(2):  # i-chunk within half
                            i0 = ih * 1024 + ic * 512
                            S = ps_S.tile([128, 1024], F32, tag="S")
                            # row-packed pair: head parity 0 on PE rows 0-63,
                            # parity 1 on rows 64-127
                            nc.tensor.matmul(
                                S[:, 0:512],
                                KT[p][0:64, j * 128:(j + 1) * 128],
                                QT[p][0:64, i0:i0 + 512],
                                start=True, stop=True, tile_position=(0, 0))
                            nc.tensor.matmul(
                                S[:, 512:1024],
                                KT[p][64:128, j * 128:(j + 1) * 128],
                                QT[p][64:128, i0:i0 + 512],
                                start=True, stop=True, tile_position=(64, 0))
                            T = tanh_pool.tile([128, 1024], F32R, tag="T")
                            nc.scalar.activation(T[:], S[:], Tanh)
                            g, a = j // 4, j % 4
                            for par in range(2):
                                lh = 2 * p + par
                                v = xv_sb[g][:, a * HALF + lh * 64:
                                             a * HALF + lh * 64 + 64]
                                nc.tensor.matmul(
                                    acc[par][ic][:],
                                    v,
                                    T[:, par * 512:(par + 1) * 512],
                                    start=(j == 0), stop=(j == NJ - 1))
                    for par in range(2):
                        for ic in range(2):
                            sl = (ih * 2 + ic) * 512
                            nc.vector.tensor_copy(stg[par][:, sl:sl + 512],
                                                  acc[par][ic][:])
                for par in range(2):
                    lh = 2 * p + par
                    nc.sync.dma_start(outT_ap[lh * 64:(lh + 1) * 64, :],
                                      stg[par][:])

    nc.compile()
    return nc


def _get_built():
    global _built
    if _built is None:
        _built = _build()
    return _built


def kernel(x, Wq, Wk):
    from concourse.bass_utils import run_bass_kernel_spmd

    x = np.asarray(x, dtype=np.float32)
    Wq = np.asarray(Wq, dtype=np.float32)
    Wk = np.asarray(Wk, dtype=np.float32)

    nc = _get_built()
    in_maps = []
    for c in range(NCORES):
        b, half = c // 2, c % 2
        sl = slice(half * HALF, (half + 1) * HALF)
        in_maps.append({
            "xT": np.ascontiguousarray(x[b].T),
            "xv": np.ascontiguousarray(x[b][:, sl]),
            "wqT": np.ascontiguousarray((SCALE * Wq[sl, :]).T.astype(np.float32)),
            "wkT": np.ascontiguousarray(Wk[sl, :].T.astype(np.float32)),
        })
    res = run_bass_kernel_spmd(nc, in_maps, core_ids=list(range(NCORES)),
                               trace=TRACE, **TRACE_KW)
    out = np.empty((B, N, DIM), np.float32)
    for c in range(NCORES):
        b, half = c // 2, c % 2
        out[b, :, half * HALF:(half + 1) * HALF] = res.results[c]["outT"].T
    if TRACE:
        kernel.last_results = res
    return out
